# revision 25
# baseline (speedup 1.0000x reference)
"""DenseGrid multi-LOD bilinear embedding lookup on 8 Trainium2 NeuronCores.

Design: dense cell-order streaming + tiny gathered overflow.

Each LOD's quad table (per interior cell: [d0(4), d1(4), g0(4), g1(4)] fp16,
where d_r = g[y1+r][x1+1]-g[y1+r][x1], g_r = g[y1+r][x1]) is laid out
partition-interleaved on the host and STREAMED contiguously into SBUF — zero
gather descriptors. Every interior cell gets a fixed budget of Kc point
slots; the host sorts points by cell and scatters the first Kc points of
each cell into the dense slot stream (fx,fx,fy,fy fp16 per slot). Points
beyond Kc go to a small per-cell-group overflow pass that fetches quads via
SWDGE dma_gather exactly like the old design, but with ~6x fewer
descriptors (the old per-group gather of all points was Q7
descriptor-generation bound at ~4 ns/idx = 1.5 ms busy).

The 5-op fp16 Horner
    r_h = g_h + fx * d_h   (h = 0, 1)
    out = r_0 + fy * (r_1 - r_0)
runs on DVE with every operand's innermost AP dim stride +-1 (fx/fy stored
duplicated as pairs), which engages the 2x_1P TENSOR_TENSOR mode — measured
~2 elem/cycle/lane vs ~1.4 for the broadcast-innermost layout.

Host does O(N) sort/index/fraction prep and O(table) re-layout only; all
per-point table-value movement happens on device.
"""
import math

import numpy as np
import concourse.bacc as bacc
import concourse.bass as bass
import concourse.mybir as mybir
import concourse.tile as tile
from concourse.bass_utils import run_bass_kernel_spmd

BASE_LOD = 4
NUM_LODS = 8
FEAT = 4
LODS = [2 ** L for L in range(BASE_LOD, BASE_LOD + NUM_LODS)]
N_POINTS = 2_000_000
N_CORES = 8
W = 32768                                # cells per overflow segment (int16)

IC = [(r - 1) * (r - 1) for r in LODS]   # interior cells per LOD
NPC = [-(-ic // N_CORES) for ic in IC]   # interior cells per core
KC = [8960, 2112, 512, 128, 36, 10, 4, 1]   # dense slots per cell
KO = [256, 256, 128, 64, 16, 8, 4, 2]       # overflow slots per group
REPL = [True, False, False, False, False, False, False, False]
NB = [NPC[l] if REPL[l] else -(-NPC[l] // 128) for l in range(NUM_LODS)]
# dense slots per partition
SPP = [NPC[l] * (KC[l] // 128) if REPL[l] else NB[l] * KC[l]
       for l in range(NUM_LODS)]
NSEG = [-(-NPC[l] // W) for l in range(NUM_LODS)]
OCOLS = [max(128, NSEG[l] * 16) for l in range(NUM_LODS)]  # ovf table cols
NI_MAX = 16384                           # idxs per gather instruction
CSMAX = 256                              # cell-rows per DVE sub-chunk
KRMAX = 256                              # k-range per DVE sub-chunk
TSLOT = 2048                             # slots/partition per DMA tile
SSLOT = 1024                             # slots/partition per DVE sub-chunk


def _ovf_moments(lam, kc, ko):
    """E and Var of ceil(max(n-kc,0)/ko) for n ~ Poisson(lam)."""
    kmax = int(lam + 10.0 * math.sqrt(lam) + 25 + kc)
    ks = np.arange(kmax + 1, dtype=np.float64)
    logfact = np.concatenate([[0.0], np.cumsum(np.log(ks[1:]))])
    pmf = np.exp(ks * math.log(lam) - lam - logfact)
    g = np.ceil(np.maximum(ks - kc, 0.0) / ko)
    e = float(np.sum(pmf * g))
    v = float(np.sum(pmf * g * g)) - e * e
    return e, max(v, 0.0)


def _make_config(scale=1.0):
    """Per-LOD per-segment overflow group capacities + gather instrs."""
    capg = []
    for l in range(NUM_LODS):
        lam = N_POINTS / IC[l]
        e1, v1 = _ovf_moments(lam, KC[l], KO[l])
        caps = []
        for s in range(NSEG[l]):
            ncell = min(W, NPC[l] - s * W)
            c_ = ncell * e1 + 6.0 * math.sqrt(max(ncell * v1, 1.0)) + 64
            caps.append(int(-(-(c_ * scale) // 128) * 128))
        capg.append(caps)
    instrs = []   # (lod, seg, group_offset, num_idxs)
    for l in range(NUM_LODS):
        off = 0
        for s in range(NSEG[l]):
            left = capg[l][s]
            while left:
                ni = min(NI_MAX, left)
                instrs.append((l, s, off, ni))
                off += ni
                left -= ni
    captot = [sum(caps) for caps in capg]
    return {"capg": capg, "captot": captot, "instrs": instrs}


def _raw_dma_gather(nc, out_ap, in_ap, idxs_ap, num_idxs, elem_size,
                    elem_step, queue_num):
    """dma_gather with elem_size_bytes below 256 (stride must be 256B mult)."""
    eng = nc.gpsimd
    stride_bytes = elem_step * mybir.dt.size(in_ap.dtype)
    assert stride_bytes % 256 == 0 and stride_bytes // 256 < 256
    assert in_ap.ap[0][0] == elem_step, in_ap.ap
    assert in_ap.ap[-1][1] == elem_size, in_ap.ap
    _in_ap = eng.lower_ap_dma(in_ap, for_custom_bir_dma=True)
    _idxs_ap = eng.lower_ap(idxs_ap)
    _out_ap = eng.lower_ap(out_ap)
    return eng.add_instruction(
        mybir.InstDMAGatherAnt(
            name=eng.bass.get_next_instruction_name(),
            ins=[*_in_ap, _idxs_ap,
                 eng.lower_val_access(eng.to_reg(num_idxs))],
            outs=[_out_ap],
            transpose=False, num_idxs=num_idxs, elem_size=elem_size,
            stride_bytes_256=stride_bytes // 256, gen_mode=0,
            single_packet=False, queue_num=queue_num,
            sbuf_tokens_per_rank=0, sbuf_free_dim_per_rank=0,
            sbuf_free_dim_pad_per_rank=0, sbuf_byte_offset=0,
        ))


def _emit_horner(nc, wk, qt, ft, ot, C, K, k_major, eng=None):
    """6-op fp16 bilinear applying per-cell quads to K packed slots.

    Quad layout per cell: [d0(4), e1(4), g00(4), e0(4)] with
    d0 = g01-g00, e0 = g10-g00, e1 = g11-g10-g01+g00:
        ma = d0*fx; mb = e1*fx; ra = ma+g00; rb = mb+e0
        out = ra + fy*rb
    Six 4-el ops instead of four 8/4-el ops: every non-broadcast operand
    is a full contiguous tile read/write (the 4-of-8 strided reads of the
    fused form measured ~1 elem/cycle vs ~2 for contiguous).
    qt: SBUF quad tile [128, C*16]; ft/ot: frc/out tiles, slot (c, k) at
    (c*K+k)*4 el, or (k*C+c)*4 when k_major (replicated-table LODs).
    frc per slot is (fx,fx,fy,fy) so every innermost AP dim is a stride-1
    pair, engaging the DVE 2x TENSOR_TENSOR mode; pre-merge 5D APs must
    stay mergeable to <= 3 free dims for the ISA.
    """
    eng = eng or nc.vector
    for k0 in range(0, K, KRMAX):
        kr = min(KRMAX, K - k0)
        n = C * kr
        tiles = [wk.tile([128, n * 4], mybir.dt.float16, tag=t, name=t)
                 for t in ("ma", "mb", "ra", "rb", "myt")]

        q3 = qt.rearrange("p (c e) -> p c e", e=16)
        if k_major:
            sl = slice(k0 * C * 4, (k0 + kr) * C * 4)
            f5 = ft[:, sl].rearrange("p (k c f b) -> p k c f b",
                                     c=C, f=2, b=2)
            o5 = ot[:, sl].rearrange("p (k c f b) -> p k c f b",
                                     c=C, f=2, b=2)
            def mk5(t):
                return t[:].rearrange("p (k c f b) -> p k c f b",
                                      c=C, f=2, b=2)
            def qb(lo):
                return q3[:, :, lo:lo + 4].rearrange(
                    "p c (f b) -> p c f b", b=2) \
                    .unsqueeze(1).broadcast_to([128, kr, C, 2, 2])
        else:
            if kr == K:
                f5 = ft.rearrange("p (c k f b) -> p c k f b",
                                  k=K, f=2, b=2)
                o5 = ot.rearrange("p (c k f b) -> p c k f b",
                                  k=K, f=2, b=2)
            else:
                # k-sliced views of a [c, K] layout only stay mergeable
                # to <= 3 free dims when there is a single cell row
                assert C == 1, (C, K)
                f5 = ft[:, k0 * 4:(k0 + kr) * 4].rearrange(
                    "p (c k f b) -> p c k f b", c=1, f=2, b=2)
                o5 = ot[:, k0 * 4:(k0 + kr) * 4].rearrange(
                    "p (c k f b) -> p c k f b", c=1, f=2, b=2)
            def mk5(t):
                return t[:].rearrange("p (c k f b) -> p c k f b",
                                      k=kr, f=2, b=2)
            def qb(lo):
                return q3[:, 0:C, lo:lo + 4].rearrange(
                    "p c (f b) -> p c f b", b=2) \
                    .unsqueeze(2).broadcast_to([128, C, kr, 2, 2])

        shp3 = [128, kr, C] if k_major else [128, C, kr]
        fx5 = f5[:, :, :, 0:1, :].broadcast_to(shp3 + [2, 2])
        fy5 = f5[:, :, :, 1:2, :].broadcast_to(shp3 + [2, 2])
        ma5, mb5, ra5, rb5, my5 = (mk5(t) for t in tiles)

        eng.tensor_mul(out=ma5, in0=qb(0), in1=fx5)
        eng.tensor_mul(out=mb5, in0=qb(4), in1=fx5)
        eng.tensor_add(out=ra5, in0=ma5, in1=qb(8))
        eng.tensor_add(out=rb5, in0=mb5, in1=qb(12))
        eng.tensor_mul(out=my5, in0=rb5, in1=fy5)
        eng.tensor_add(out=o5, in0=ra5, in1=my5)


def _build_program(cfg):
    captot = cfg["captot"]
    nc = bacc.Bacc(None, target_bir_lowering=False, num_swdge_queues=4)
    with tile.TileContext(nc) as tc:
        with tc.tile_pool(name="dram", bufs=1, space="DRAM") as dram, \
             tc.tile_pool(name="ov", bufs=1) as ov, \
             tc.tile_pool(name="qp", bufs=2) as qp, \
             tc.tile_pool(name="fp", bufs=3) as fp, \
             tc.tile_pool(name="op", bufs=2) as op, \
             tc.tile_pool(name="wk", bufs=1) as wk:
            tabd = [dram.tile([128, NB[l] * 16], mybir.dt.float16,
                              kind="ExternalInput", name=f"tabd_{l}")
                    for l in range(NUM_LODS)]
            frcd = [dram.tile([128, SPP[l] * 4], mybir.dt.float16,
                              kind="ExternalInput", name=f"frcd_{l}")
                    for l in range(NUM_LODS)]
            outd = [dram.tile([128, SPP[l] * 4], mybir.dt.float16,
                              kind="ExternalOutput", name=f"outd_{l}")
                    for l in range(NUM_LODS)]
            tabo = [dram.tile([min(W, NPC[l]), OCOLS[l]], mybir.dt.float16,
                              kind="ExternalInput", name=f"tabo_{l}")
                    for l in range(NUM_LODS)]
            idxo = [dram.tile([128, captot[l] // 16], mybir.dt.int16,
                              kind="ExternalInput", name=f"idxo_{l}")
                    for l in range(NUM_LODS)]
            frco = [dram.tile([128, captot[l] * KO[l] // 128 * 4],
                              mybir.dt.float16,
                              kind="ExternalInput", name=f"frco_{l}")
                    for l in range(NUM_LODS)]
            outo = [dram.tile([128, captot[l] * KO[l] // 128 * 4],
                              mybir.dt.float16,
                              kind="ExternalOutput", name=f"outo_{l}")
                    for l in range(NUM_LODS)]

            # ---- dense streamed chunk emitter ---------------------------
            def emit_dense(l):
                if REPL[l]:
                    # replicated quad table, stream order (k, c)
                    qt = qp.tile([128, NB[l] * 16], mybir.dt.float16,
                                 tag="qtr")
                    nc.sync.dma_start(out=qt[:], in_=tabd[l][:])
                    kcp = KC[l] // 128
                    C = NPC[l]
                    kchunk = max(1, SSLOT // C)
                    ktile = 2 * kchunk
                    for t0 in range(0, kcp, ktile):
                        tr = min(ktile, kcp - t0)
                        n = C * tr
                        ft = fp.tile([128, n * 4], mybir.dt.float16,
                                     tag="ft")
                        nc.scalar.dma_start(
                            out=ft[:],
                            in_=frcd[l][:, t0 * C * 4:(t0 + tr) * C * 4])
                        ot = op.tile([128, n * 4], mybir.dt.float16,
                                     tag="ot")
                        for k0 in range(0, tr, kchunk):
                            kr = min(kchunk, tr - k0)
                            _emit_horner(
                                nc, wk, qt[:],
                                ft[:, k0 * C * 4:(k0 + kr) * C * 4],
                                ot[:, k0 * C * 4:(k0 + kr) * C * 4],
                                C, kr, True)
                        nc.sync.dma_start(
                            out=outd[l][:, t0 * C * 4:(t0 + tr) * C * 4],
                            in_=ot[:])
                elif KC[l] > SSLOT:
                    # one cell row per partition, k-chunked (L1)
                    K = KC[l]
                    qt = qp.tile([128, 16], mybir.dt.float16, tag="qt1")
                    nc.sync.dma_start(out=qt[:], in_=tabd[l][:])
                    for k0 in range(0, K, TSLOT):
                        kb = min(TSLOT, K - k0)
                        ft = fp.tile([128, kb * 4], mybir.dt.float16,
                                     tag="ft")
                        nc.scalar.dma_start(
                            out=ft[:], in_=frcd[l][:, k0 * 4:(k0 + kb) * 4])
                        ot = op.tile([128, kb * 4], mybir.dt.float16,
                                     tag="ot")
                        _emit_horner(nc, wk, qt[:], ft[:], ot[:],
                                     1, kb, False)
                        nc.sync.dma_start(
                            out=outd[l][:, k0 * 4:(k0 + kb) * 4], in_=ot[:])
                else:
                    K = KC[l]
                    cchunk = 1 if K > KRMAX else \
                        max(1, min(CSMAX, SSLOT // K))
                    ctile = 2 * cchunk
                    for t0 in range(0, NB[l], ctile):
                        ts = min(ctile, NB[l] - t0)
                        qt = qp.tile([128, ts * 16], mybir.dt.float16,
                                     tag="qt")
                        nc.sync.dma_start(
                            out=qt[:],
                            in_=tabd[l][:, t0 * 16:(t0 + ts) * 16])
                        ft = fp.tile([128, ts * K * 4], mybir.dt.float16,
                                     tag="ft")
                        nc.scalar.dma_start(
                            out=ft[:],
                            in_=frcd[l][:, t0 * K * 4:(t0 + ts) * K * 4])
                        ot = op.tile([128, ts * K * 4], mybir.dt.float16,
                                     tag="ot")
                        for c0 in range(0, ts, cchunk):
                            cs = min(cchunk, ts - c0)
                            _emit_horner(
                                nc, wk,
                                qt[:, c0 * 16:(c0 + cs) * 16],
                                ft[:, c0 * K * 4:(c0 + cs) * K * 4],
                                ot[:, c0 * K * 4:(c0 + cs) * K * 4],
                                cs, K, False)
                        nc.sync.dma_start(
                            out=outd[l][:, t0 * K * 4:(t0 + ts) * K * 4],
                            in_=ot[:])

            emit_dense(0)

            # ---- phase 1: overflow idx/frc loads + all gathers ----------
            it = []
            fot = []
            qot = []
            for l in range(NUM_LODS):
                t = ov.tile([128, captot[l] // 16], mybir.dt.int16,
                            tag=f"it{l}")
                nc.sync.dma_start(out=t[:], in_=idxo[l][:])
                it.append(t)
                t = ov.tile([128, captot[l] * KO[l] // 128 * 4],
                            mybir.dt.float16, tag=f"fo{l}")
                nc.scalar.dma_start(out=t[:], in_=frco[l][:])
                fot.append(t)
                qo_t = ov.tile([128, captot[l] // 128 * 16],
                               mybir.dt.float16, tag=f"qo{l}",
                               name=f"qo{l}")
                qot.append(qo_t)
            qn = 0
            for (l, s, off, ni) in cfg["instrs"]:
                _raw_dma_gather(
                    nc,
                    out_ap=qot[l][:, off // 128 * 16:(off + ni) // 128 * 16]
                    .rearrange("p (c e) -> p c e", e=16),
                    in_ap=tabo[l][:][:, 16 * s:16 * s + 16],
                    idxs_ap=it[l][:, off // 16:(off + ni) // 16],
                    num_idxs=ni, elem_size=16, elem_step=OCOLS[l],
                    queue_num=qn % 4)
                qn += 1

            for l in range(1, NUM_LODS):
                emit_dense(l)

            # ---- phase 3: overflow applies (gpsimd, own pools) ----------
            for l in range(NUM_LODS):
                rows = captot[l] // 128
                K = KO[l]
                cchunk = max(1, min(CSMAX, 1024 // K))
                for c0 in range(0, rows, cchunk):
                    cs = min(cchunk, rows - c0)
                    ot = op.tile([128, cs * K * 4], mybir.dt.float16,
                                 tag="ot")
                    _emit_horner(nc, wk,
                                 qot[l][:, c0 * 16:(c0 + cs) * 16],
                                 fot[l][:, c0 * K * 4:(c0 + cs) * K * 4],
                                 ot[:], cs, K, False)
                    nc.scalar.dma_start(
                        out=outo[l][:, c0 * K * 4:(c0 + cs) * K * 4],
                        in_=ot[:])
    nc.compile()
    names = {
        "tabd": [t.name for t in tabd], "frcd": [t.name for t in frcd],
        "outd": [t.name for t in outd], "tabo": [t.name for t in tabo],
        "idxo": [t.name for t in idxo], "frco": [t.name for t in frco],
        "outo": [t.name for t in outo],
    }
    return nc, names


_cache = {}


def _quads(g, l):
    """Interior-cell quad array [(res-1)^2, 16] fp16."""
    res = LODS[l]
    g3 = np.asarray(g, dtype=np.float32).reshape(res, res, FEAT)
    q = np.empty((res - 1, res - 1, 16), dtype=np.float16)
    d0 = g3[:-1, 1:] - g3[:-1, :-1]
    q[:, :, 0:4] = d0
    q[:, :, 4:8] = (g3[1:, 1:] - g3[1:, :-1]) - d0
    q[:, :, 8:12] = g3[:-1, :-1]
    q[:, :, 12:16] = g3[1:, :-1] - g3[:-1, :-1]
    return q.reshape(-1, 16)


def _dense_table(qf, l, c):
    """Per-core dense streamed table [128, NB*16]."""
    npc = NPC[l]
    sl = qf[c * npc:min((c + 1) * npc, IC[l])]
    if REPL[l]:
        flat = np.zeros(npc * 16, dtype=np.float16)
        flat[:sl.size] = sl.reshape(-1)
        return np.ascontiguousarray(
            np.broadcast_to(flat[None, :], (128, npc * 16)))
    nb = NB[l]
    arr = np.zeros((nb * 128, 16), dtype=np.float16)
    arr[:len(sl)] = sl
    return np.ascontiguousarray(
        arr.reshape(nb, 128, 16).transpose(1, 0, 2)).reshape(128, nb * 16)


def _ovf_table(qf, l, c):
    """Per-core overflow gather table [min(W,NPC), OCOLS]."""
    npc = NPC[l]
    sl = qf[c * npc:min((c + 1) * npc, IC[l])]
    rows = min(W, npc)
    out = np.zeros((rows, OCOLS[l]), dtype=np.float16)
    for s in range(NSEG[l]):
        seg = sl[s * W:(s + 1) * W]
        out[:len(seg), 16 * s:16 * s + 16] = seg
    return out


def _streams(x, l, cfg):
    """Host: per-core dense + overflow streams for LOD l."""
    res = LODS[l]
    R1 = res - 1
    Kc, Ko = KC[l], KO[l]
    npc = NPC[l]
    caps = cfg["capg"][l]
    captot = int(np.sum(caps))
    spp = SPP[l]
    xs = x[:, 0] * np.float32(R1)
    ys = x[:, 1] * np.float32(R1)
    hi = np.float32(R1 - 1e-05)
    x1 = np.floor(np.clip(xs, 0, hi)).astype(np.int32)
    y1 = np.floor(np.clip(ys, 0, hi)).astype(np.int32)
    fx = (xs - x1.astype(np.float32)).astype(np.float16)
    fy = (ys - y1.astype(np.float32)).astype(np.float16)
    ic = y1 * R1 + x1
    order = np.argsort(ic, kind="stable")
    sic = ic[order]
    cb = np.searchsorted(sic, np.arange(N_CORES + 1, dtype=np.int64) * npc)
    per_core = []
    for c in range(N_CORES):
        o_c = order[cb[c]:cb[c + 1]]
        lid = (sic[cb[c]:cb[c + 1]] - c * npc).astype(np.int64)
        n = len(lid)
        if n:
            newc = np.empty(n, dtype=bool)
            newc[0] = True
            newc[1:] = lid[1:] != lid[:-1]
            run_start = np.maximum.accumulate(
                np.where(newc, np.arange(n), 0))
            rank = np.arange(n) - run_start
        else:
            rank = np.zeros(0, dtype=np.int64)
        fxc = fx[o_c]
        fyc = fy[o_c]

        dm = rank < Kc
        dlid, dr = lid[dm], rank[dm]
        if REPL[l]:
            p = dr % 128
            fo = (dr // 128) * npc + dlid
        else:
            p = dlid % 128
            fo = (dlid // 128) * Kc + dr
        dpos = p * spp + fo
        frcd_a = np.zeros(128 * spp * 4, dtype=np.float16)
        b4 = dpos * 4
        frcd_a[b4] = fxc[dm]
        frcd_a[b4 + 1] = fxc[dm]
        frcd_a[b4 + 2] = fyc[dm]
        frcd_a[b4 + 3] = fyc[dm]

        # overflow
        ovm = ~dm
        olid = lid[ovm]
        orank = rank[ovm] - Kc
        is_g = (orank % Ko) == 0
        gidx = np.cumsum(is_g) - 1
        glid = olid[is_g]
        seg_of_g = (glid >> 15).astype(np.int64)
        gs = np.searchsorted(seg_of_g, np.arange(len(caps)))
        gs = np.append(gs, len(seg_of_g))
        if np.any(np.diff(gs) > np.asarray(caps)):
            raise RuntimeError(
                f"ovf overflow LOD{l} core{c}: {np.diff(gs)} caps {caps}")
        base = np.concatenate([[0], np.cumsum(caps)])[:-1]
        gpos = base[seg_of_g] + (np.arange(len(seg_of_g)) - gs[seg_of_g])
        idx_s = np.zeros(captot, dtype=np.int16)
        idx_s[gpos] = (glid & 32767).astype(np.int16)
        gp = gpos[gidx] if len(gidx) else np.zeros(0, dtype=np.int64)
        opos = (gp // 128) * Ko + orank % Ko + (gp % 128) * \
            (captot // 128 * Ko)
        osl = captot // 128 * Ko
        frco_a = np.zeros(128 * osl * 4, dtype=np.float16)
        b4 = opos * 4
        frco_a[b4] = fxc[ovm]
        frco_a[b4 + 1] = fxc[ovm]
        frco_a[b4 + 2] = fyc[ovm]
        frco_a[b4 + 3] = fyc[ovm]

        per_core.append({
            "frcd": frcd_a.reshape(128, spp * 4),
            "idxo": np.ascontiguousarray(
                np.tile(idx_s.reshape(-1, 16).T, (8, 1))),
            "frco": frco_a.reshape(128, osl * 4),
            "o_dense": o_c[dm], "pos_dense": dpos,
            "o_ovf": o_c[ovm], "pos_ovf": opos,
        })
    return per_core


def kernel(**inputs):
    x = np.asarray(inputs["x"], dtype=np.float32)
    assert x.shape == (N_POINTS, 2), x.shape

    qfs = [_quads(inputs[f"grid_{l}"], l) for l in range(NUM_LODS)]

    scale = 1.0
    for _attempt in range(4):
        cfg = _make_config(scale)
        key = tuple(cfg["captot"])
        if key not in _cache:
            _cache[key] = _build_program(cfg)
        nc, names = _cache[key]
        try:
            streams = [_streams(x, l, cfg) for l in range(NUM_LODS)]
            break
        except RuntimeError:
            scale *= 1.5
    else:
        raise RuntimeError("stream capacity overflow")

    in_maps = []
    for c in range(N_CORES):
        m = {}
        for l in range(NUM_LODS):
            m[names["tabd"][l]] = _dense_table(qfs[l], l, c)
            m[names["tabo"][l]] = _ovf_table(qfs[l], l, c)
            s = streams[l][c]
            m[names["frcd"][l]] = s["frcd"]
            m[names["idxo"][l]] = s["idxo"]
            m[names["frco"][l]] = s["frco"]
        in_maps.append(m)

    res = run_bass_kernel_spmd(nc, in_maps, core_ids=list(range(N_CORES)))

    out = np.empty((N_POINTS, NUM_LODS * FEAT), dtype=np.float32)
    for l in range(NUM_LODS):
        for c in range(N_CORES):
            s = streams[l][c]
            od = np.asarray(res.results[c][names["outd"][l]]).reshape(-1, 4)
            out[s["o_dense"], l * FEAT:(l + 1) * FEAT] = \
                od[s["pos_dense"]].astype(np.float32)
            if len(s["o_ovf"]):
                oo = np.asarray(
                    res.results[c][names["outo"][l]]).reshape(-1, 4)
                out[s["o_ovf"], l * FEAT:(l + 1) * FEAT] = \
                    oo[s["pos_ovf"]].astype(np.float32)
    return out


# revision 26
# speedup vs baseline: 1.0149x; 1.0149x over previous
"""DenseGrid multi-LOD bilinear embedding lookup on 8 Trainium2 NeuronCores.

Design: dense cell-order streaming + tiny gathered overflow.

Each LOD's quad table (per interior cell: [d0(4), d1(4), g0(4), g1(4)] fp16,
where d_r = g[y1+r][x1+1]-g[y1+r][x1], g_r = g[y1+r][x1]) is laid out
partition-interleaved on the host and STREAMED contiguously into SBUF — zero
gather descriptors. Every interior cell gets a fixed budget of Kc point
slots; the host sorts points by cell and scatters the first Kc points of
each cell into the dense slot stream (fx,fx,fy,fy fp16 per slot). Points
beyond Kc go to a small per-cell-group overflow pass that fetches quads via
SWDGE dma_gather exactly like the old design, but with ~6x fewer
descriptors (the old per-group gather of all points was Q7
descriptor-generation bound at ~4 ns/idx = 1.5 ms busy).

The 5-op fp16 Horner
    r_h = g_h + fx * d_h   (h = 0, 1)
    out = r_0 + fy * (r_1 - r_0)
runs on DVE with every operand's innermost AP dim stride +-1 (fx/fy stored
duplicated as pairs), which engages the 2x_1P TENSOR_TENSOR mode — measured
~2 elem/cycle/lane vs ~1.4 for the broadcast-innermost layout.

Host does O(N) sort/index/fraction prep and O(table) re-layout only; all
per-point table-value movement happens on device.
"""
import math

import numpy as np
import concourse.bacc as bacc
import concourse.bass as bass
import concourse.mybir as mybir
import concourse.tile as tile
from concourse.bass_utils import run_bass_kernel_spmd

BASE_LOD = 4
NUM_LODS = 8
FEAT = 4
LODS = [2 ** L for L in range(BASE_LOD, BASE_LOD + NUM_LODS)]
N_POINTS = 2_000_000
N_CORES = 8
W = 32768                                # cells per overflow segment (int16)

IC = [(r - 1) * (r - 1) for r in LODS]   # interior cells per LOD
NPC = [-(-ic // N_CORES) for ic in IC]   # interior cells per core
KC = [8960, 2112, 512, 128, 36, 10, 4, 1]   # dense slots per cell
KO = [256, 256, 128, 64, 16, 8, 4, 2]       # overflow slots per group
REPL = [True, False, False, False, False, False, False, False]
NB = [NPC[l] if REPL[l] else -(-NPC[l] // 128) for l in range(NUM_LODS)]
# dense slots per partition
SPP = [NPC[l] * (KC[l] // 128) if REPL[l] else NB[l] * KC[l]
       for l in range(NUM_LODS)]
NSEG = [-(-NPC[l] // W) for l in range(NUM_LODS)]
OCOLS = [max(128, NSEG[l] * 16) for l in range(NUM_LODS)]  # ovf table cols
NI_MAX = 16384                           # idxs per gather instruction
CSMAX = 256                              # cell-rows per DVE sub-chunk
KRMAX = 256                              # k-range per DVE sub-chunk
TSLOT = 2048                             # slots/partition per DMA tile
SSLOT = 1024                             # slots/partition per DVE sub-chunk


def _ovf_moments(lam, kc, ko):
    """E and Var of ceil(max(n-kc,0)/ko) for n ~ Poisson(lam)."""
    kmax = int(lam + 10.0 * math.sqrt(lam) + 25 + kc)
    ks = np.arange(kmax + 1, dtype=np.float64)
    logfact = np.concatenate([[0.0], np.cumsum(np.log(ks[1:]))])
    pmf = np.exp(ks * math.log(lam) - lam - logfact)
    g = np.ceil(np.maximum(ks - kc, 0.0) / ko)
    e = float(np.sum(pmf * g))
    v = float(np.sum(pmf * g * g)) - e * e
    return e, max(v, 0.0)


def _make_config(scale=1.0):
    """Per-LOD per-segment overflow group capacities + gather instrs."""
    capg = []
    for l in range(NUM_LODS):
        lam = N_POINTS / IC[l]
        e1, v1 = _ovf_moments(lam, KC[l], KO[l])
        caps = []
        for s in range(NSEG[l]):
            ncell = min(W, NPC[l] - s * W)
            c_ = ncell * e1 + 6.0 * math.sqrt(max(ncell * v1, 1.0)) + 64
            caps.append(int(-(-(c_ * scale) // 128) * 128))
        capg.append(caps)
    instrs = []   # (lod, seg, group_offset, num_idxs)
    for l in range(NUM_LODS):
        off = 0
        for s in range(NSEG[l]):
            left = capg[l][s]
            while left:
                ni = min(NI_MAX, left)
                instrs.append((l, s, off, ni))
                off += ni
                left -= ni
    captot = [sum(caps) for caps in capg]
    return {"capg": capg, "captot": captot, "instrs": instrs}


def _raw_dma_gather(nc, out_ap, in_ap, idxs_ap, num_idxs, elem_size,
                    elem_step, queue_num):
    """dma_gather with elem_size_bytes below 256 (stride must be 256B mult)."""
    eng = nc.gpsimd
    stride_bytes = elem_step * mybir.dt.size(in_ap.dtype)
    assert stride_bytes % 256 == 0 and stride_bytes // 256 < 256
    assert in_ap.ap[0][0] == elem_step, in_ap.ap
    assert in_ap.ap[-1][1] == elem_size, in_ap.ap
    _in_ap = eng.lower_ap_dma(in_ap, for_custom_bir_dma=True)
    _idxs_ap = eng.lower_ap(idxs_ap)
    _out_ap = eng.lower_ap(out_ap)
    return eng.add_instruction(
        mybir.InstDMAGatherAnt(
            name=eng.bass.get_next_instruction_name(),
            ins=[*_in_ap, _idxs_ap,
                 eng.lower_val_access(eng.to_reg(num_idxs))],
            outs=[_out_ap],
            transpose=False, num_idxs=num_idxs, elem_size=elem_size,
            stride_bytes_256=stride_bytes // 256, gen_mode=0,
            single_packet=False, queue_num=queue_num,
            sbuf_tokens_per_rank=0, sbuf_free_dim_per_rank=0,
            sbuf_free_dim_pad_per_rank=0, sbuf_byte_offset=0,
        ))


def _emit_horner(nc, wk, qt, ft, ot, C, K, k_major, eng=None):
    """6-op fp16 bilinear applying per-cell quads to K packed slots.

    Quad layout per cell: [d0(4), e1(4), g00(4), e0(4)] with
    d0 = g01-g00, e0 = g10-g00, e1 = g11-g10-g01+g00:
        ma = d0*fx; mb = e1*fx; ra = ma+g00; rb = mb+e0
        out = ra + fy*rb
    Six 4-el ops instead of four 8/4-el ops: every non-broadcast operand
    is a full contiguous tile read/write (the 4-of-8 strided reads of the
    fused form measured ~1 elem/cycle vs ~2 for contiguous).
    qt: SBUF quad tile [128, C*16]; ft/ot: frc/out tiles, slot (c, k) at
    (c*K+k)*4 el, or (k*C+c)*4 when k_major (replicated-table LODs).
    frc per slot is (fx,fx,fy,fy) so every innermost AP dim is a stride-1
    pair, engaging the DVE 2x TENSOR_TENSOR mode; pre-merge 5D APs must
    stay mergeable to <= 3 free dims for the ISA.
    """
    eng = eng or nc.vector
    for k0 in range(0, K, KRMAX):
        kr = min(KRMAX, K - k0)
        n = C * kr
        tiles = [wk.tile([128, n * 4], mybir.dt.float16, tag=t, name=t)
                 for t in ("ma", "mb", "ra", "rb", "myt")]

        q3 = qt.rearrange("p (c e) -> p c e", e=16)
        if k_major:
            sl = slice(k0 * C * 4, (k0 + kr) * C * 4)
            f5 = ft[:, sl].rearrange("p (k c f b) -> p k c f b",
                                     c=C, f=2, b=2)
            o5 = ot[:, sl].rearrange("p (k c f b) -> p k c f b",
                                     c=C, f=2, b=2)
            def mk5(t):
                return t[:].rearrange("p (k c f b) -> p k c f b",
                                      c=C, f=2, b=2)
            def qb(lo):
                return q3[:, :, lo:lo + 4].rearrange(
                    "p c (f b) -> p c f b", b=2) \
                    .unsqueeze(1).broadcast_to([128, kr, C, 2, 2])
        else:
            if kr == K:
                f5 = ft.rearrange("p (c k f b) -> p c k f b",
                                  k=K, f=2, b=2)
                o5 = ot.rearrange("p (c k f b) -> p c k f b",
                                  k=K, f=2, b=2)
            else:
                # k-sliced views of a [c, K] layout only stay mergeable
                # to <= 3 free dims when there is a single cell row
                assert C == 1, (C, K)
                f5 = ft[:, k0 * 4:(k0 + kr) * 4].rearrange(
                    "p (c k f b) -> p c k f b", c=1, f=2, b=2)
                o5 = ot[:, k0 * 4:(k0 + kr) * 4].rearrange(
                    "p (c k f b) -> p c k f b", c=1, f=2, b=2)
            def mk5(t):
                return t[:].rearrange("p (c k f b) -> p c k f b",
                                      k=kr, f=2, b=2)
            def qb(lo):
                return q3[:, 0:C, lo:lo + 4].rearrange(
                    "p c (f b) -> p c f b", b=2) \
                    .unsqueeze(2).broadcast_to([128, C, kr, 2, 2])

        shp3 = [128, kr, C] if k_major else [128, C, kr]
        fx5 = f5[:, :, :, 0:1, :].broadcast_to(shp3 + [2, 2])
        fy5 = f5[:, :, :, 1:2, :].broadcast_to(shp3 + [2, 2])
        ma5, mb5, ra5, rb5, my5 = (mk5(t) for t in tiles)

        eng.tensor_mul(out=ma5, in0=qb(0), in1=fx5)
        eng.tensor_mul(out=mb5, in0=qb(4), in1=fx5)
        eng.tensor_add(out=ra5, in0=ma5, in1=qb(8))
        eng.tensor_add(out=rb5, in0=mb5, in1=qb(12))
        eng.tensor_mul(out=my5, in0=rb5, in1=fy5)
        eng.tensor_add(out=o5, in0=ra5, in1=my5)


def _build_program(cfg):
    captot = cfg["captot"]
    nc = bacc.Bacc(None, target_bir_lowering=False, num_swdge_queues=4)
    with tile.TileContext(nc) as tc:
        with tc.tile_pool(name="dram", bufs=1, space="DRAM") as dram, \
             tc.tile_pool(name="ov", bufs=1) as ov, \
             tc.tile_pool(name="qp", bufs=3) as qp, \
             tc.tile_pool(name="fp", bufs=2) as fp, \
             tc.tile_pool(name="op", bufs=2) as op, \
             tc.tile_pool(name="wk", bufs=1) as wk:
            tabd = [dram.tile([128, NB[l] * 16], mybir.dt.float16,
                              kind="ExternalInput", name=f"tabd_{l}")
                    for l in range(NUM_LODS)]
            frcd = [dram.tile([128, SPP[l] * 4], mybir.dt.float16,
                              kind="ExternalInput", name=f"frcd_{l}")
                    for l in range(NUM_LODS)]
            outd = [dram.tile([128, SPP[l] * 4], mybir.dt.float16,
                              kind="ExternalOutput", name=f"outd_{l}")
                    for l in range(NUM_LODS)]
            tabo = [dram.tile([min(W, NPC[l]), OCOLS[l]], mybir.dt.float16,
                              kind="ExternalInput", name=f"tabo_{l}")
                    for l in range(NUM_LODS)]
            idxo = [dram.tile([128, captot[l] // 16], mybir.dt.int16,
                              kind="ExternalInput", name=f"idxo_{l}")
                    for l in range(NUM_LODS)]
            frco = [dram.tile([128, captot[l] * KO[l] // 128 * 4],
                              mybir.dt.float16,
                              kind="ExternalInput", name=f"frco_{l}")
                    for l in range(NUM_LODS)]
            outo = [dram.tile([128, captot[l] * KO[l] // 128 * 4],
                              mybir.dt.float16,
                              kind="ExternalOutput", name=f"outo_{l}")
                    for l in range(NUM_LODS)]

            # ---- dense streamed chunk emitter ---------------------------
            def emit_dense(l):
                if REPL[l]:
                    # replicated quad table, stream order (k, c)
                    qt = qp.tile([128, NB[l] * 16], mybir.dt.float16,
                                 tag="qtr")
                    nc.sync.dma_start(out=qt[:], in_=tabd[l][:])
                    kcp = KC[l] // 128
                    C = NPC[l]
                    kchunk = max(1, SSLOT // C)
                    ktile = 2 * kchunk
                    for t0 in range(0, kcp, ktile):
                        tr = min(ktile, kcp - t0)
                        n = C * tr
                        ft = fp.tile([128, n * 4], mybir.dt.float16,
                                     tag="ft")
                        nc.scalar.dma_start(
                            out=ft[:],
                            in_=frcd[l][:, t0 * C * 4:(t0 + tr) * C * 4])
                        ot = op.tile([128, n * 4], mybir.dt.float16,
                                     tag="ot")
                        for k0 in range(0, tr, kchunk):
                            kr = min(kchunk, tr - k0)
                            _emit_horner(
                                nc, wk, qt[:],
                                ft[:, k0 * C * 4:(k0 + kr) * C * 4],
                                ot[:, k0 * C * 4:(k0 + kr) * C * 4],
                                C, kr, True)
                        nc.sync.dma_start(
                            out=outd[l][:, t0 * C * 4:(t0 + tr) * C * 4],
                            in_=ot[:])
                elif KC[l] > SSLOT:
                    # one cell row per partition, k-chunked (L1)
                    K = KC[l]
                    qt = qp.tile([128, 16], mybir.dt.float16, tag="qt1")
                    nc.sync.dma_start(out=qt[:], in_=tabd[l][:])
                    for k0 in range(0, K, TSLOT):
                        kb = min(TSLOT, K - k0)
                        ft = fp.tile([128, kb * 4], mybir.dt.float16,
                                     tag="ft")
                        nc.scalar.dma_start(
                            out=ft[:], in_=frcd[l][:, k0 * 4:(k0 + kb) * 4])
                        ot = op.tile([128, kb * 4], mybir.dt.float16,
                                     tag="ot")
                        _emit_horner(nc, wk, qt[:], ft[:], ot[:],
                                     1, kb, False)
                        nc.sync.dma_start(
                            out=outd[l][:, k0 * 4:(k0 + kb) * 4], in_=ot[:])
                else:
                    K = KC[l]
                    cchunk = 1 if K > KRMAX else \
                        max(1, min(CSMAX, SSLOT // K))
                    ctile = 2 * cchunk
                    for t0 in range(0, NB[l], ctile):
                        ts = min(ctile, NB[l] - t0)
                        qt = qp.tile([128, ts * 16], mybir.dt.float16,
                                     tag="qt")
                        nc.sync.dma_start(
                            out=qt[:],
                            in_=tabd[l][:, t0 * 16:(t0 + ts) * 16])
                        ft = fp.tile([128, ts * K * 4], mybir.dt.float16,
                                     tag="ft")
                        nc.scalar.dma_start(
                            out=ft[:],
                            in_=frcd[l][:, t0 * K * 4:(t0 + ts) * K * 4])
                        ot = op.tile([128, ts * K * 4], mybir.dt.float16,
                                     tag="ot")
                        for c0 in range(0, ts, cchunk):
                            cs = min(cchunk, ts - c0)
                            _emit_horner(
                                nc, wk,
                                qt[:, c0 * 16:(c0 + cs) * 16],
                                ft[:, c0 * K * 4:(c0 + cs) * K * 4],
                                ot[:, c0 * K * 4:(c0 + cs) * K * 4],
                                cs, K, False)
                        oeng = nc.scalar if (t0 // max(ctile, 1)) % 2 \
                            else nc.sync
                        oeng.dma_start(
                            out=outd[l][:, t0 * K * 4:(t0 + ts) * K * 4],
                            in_=ot[:])

            emit_dense(0)

            # ---- phase 1: overflow idx/frc loads + all gathers ----------
            it = []
            fot = []
            qot = []
            for l in range(NUM_LODS):
                t = ov.tile([128, captot[l] // 16], mybir.dt.int16,
                            tag=f"it{l}")
                nc.sync.dma_start(out=t[:], in_=idxo[l][:])
                it.append(t)
                t = ov.tile([128, captot[l] * KO[l] // 128 * 4],
                            mybir.dt.float16, tag=f"fo{l}")
                nc.scalar.dma_start(out=t[:], in_=frco[l][:])
                fot.append(t)
                qo_t = ov.tile([128, captot[l] // 128 * 16],
                               mybir.dt.float16, tag=f"qo{l}",
                               name=f"qo{l}")
                qot.append(qo_t)
            qn = 0
            for (l, s, off, ni) in cfg["instrs"]:
                _raw_dma_gather(
                    nc,
                    out_ap=qot[l][:, off // 128 * 16:(off + ni) // 128 * 16]
                    .rearrange("p (c e) -> p c e", e=16),
                    in_ap=tabo[l][:][:, 16 * s:16 * s + 16],
                    idxs_ap=it[l][:, off // 16:(off + ni) // 16],
                    num_idxs=ni, elem_size=16, elem_step=OCOLS[l],
                    queue_num=qn % 4)
                qn += 1

            for l in range(1, NUM_LODS):
                emit_dense(l)

            # ---- phase 3: overflow applies (gpsimd, own pools) ----------
            for l in range(NUM_LODS):
                rows = captot[l] // 128
                K = KO[l]
                cchunk = max(1, min(CSMAX, 1024 // K))
                for c0 in range(0, rows, cchunk):
                    cs = min(cchunk, rows - c0)
                    ot = op.tile([128, cs * K * 4], mybir.dt.float16,
                                 tag="ot")
                    _emit_horner(nc, wk,
                                 qot[l][:, c0 * 16:(c0 + cs) * 16],
                                 fot[l][:, c0 * K * 4:(c0 + cs) * K * 4],
                                 ot[:], cs, K, False)
                    nc.scalar.dma_start(
                        out=outo[l][:, c0 * K * 4:(c0 + cs) * K * 4],
                        in_=ot[:])
    nc.compile()
    names = {
        "tabd": [t.name for t in tabd], "frcd": [t.name for t in frcd],
        "outd": [t.name for t in outd], "tabo": [t.name for t in tabo],
        "idxo": [t.name for t in idxo], "frco": [t.name for t in frco],
        "outo": [t.name for t in outo],
    }
    return nc, names


_cache = {}


def _quads(g, l):
    """Interior-cell quad array [(res-1)^2, 16] fp16."""
    res = LODS[l]
    g3 = np.asarray(g, dtype=np.float32).reshape(res, res, FEAT)
    q = np.empty((res - 1, res - 1, 16), dtype=np.float16)
    d0 = g3[:-1, 1:] - g3[:-1, :-1]
    q[:, :, 0:4] = d0
    q[:, :, 4:8] = (g3[1:, 1:] - g3[1:, :-1]) - d0
    q[:, :, 8:12] = g3[:-1, :-1]
    q[:, :, 12:16] = g3[1:, :-1] - g3[:-1, :-1]
    return q.reshape(-1, 16)


def _dense_table(qf, l, c):
    """Per-core dense streamed table [128, NB*16]."""
    npc = NPC[l]
    sl = qf[c * npc:min((c + 1) * npc, IC[l])]
    if REPL[l]:
        flat = np.zeros(npc * 16, dtype=np.float16)
        flat[:sl.size] = sl.reshape(-1)
        return np.ascontiguousarray(
            np.broadcast_to(flat[None, :], (128, npc * 16)))
    nb = NB[l]
    arr = np.zeros((nb * 128, 16), dtype=np.float16)
    arr[:len(sl)] = sl
    return np.ascontiguousarray(
        arr.reshape(nb, 128, 16).transpose(1, 0, 2)).reshape(128, nb * 16)


def _ovf_table(qf, l, c):
    """Per-core overflow gather table [min(W,NPC), OCOLS]."""
    npc = NPC[l]
    sl = qf[c * npc:min((c + 1) * npc, IC[l])]
    rows = min(W, npc)
    out = np.zeros((rows, OCOLS[l]), dtype=np.float16)
    for s in range(NSEG[l]):
        seg = sl[s * W:(s + 1) * W]
        out[:len(seg), 16 * s:16 * s + 16] = seg
    return out


def _streams(x, l, cfg):
    """Host: per-core dense + overflow streams for LOD l."""
    res = LODS[l]
    R1 = res - 1
    Kc, Ko = KC[l], KO[l]
    npc = NPC[l]
    caps = cfg["capg"][l]
    captot = int(np.sum(caps))
    spp = SPP[l]
    xs = x[:, 0] * np.float32(R1)
    ys = x[:, 1] * np.float32(R1)
    hi = np.float32(R1 - 1e-05)
    x1 = np.floor(np.clip(xs, 0, hi)).astype(np.int32)
    y1 = np.floor(np.clip(ys, 0, hi)).astype(np.int32)
    fx = (xs - x1.astype(np.float32)).astype(np.float16)
    fy = (ys - y1.astype(np.float32)).astype(np.float16)
    ic = y1 * R1 + x1
    order = np.argsort(ic, kind="stable")
    sic = ic[order]
    cb = np.searchsorted(sic, np.arange(N_CORES + 1, dtype=np.int64) * npc)
    per_core = []
    for c in range(N_CORES):
        o_c = order[cb[c]:cb[c + 1]]
        lid = (sic[cb[c]:cb[c + 1]] - c * npc).astype(np.int64)
        n = len(lid)
        if n:
            newc = np.empty(n, dtype=bool)
            newc[0] = True
            newc[1:] = lid[1:] != lid[:-1]
            run_start = np.maximum.accumulate(
                np.where(newc, np.arange(n), 0))
            rank = np.arange(n) - run_start
        else:
            rank = np.zeros(0, dtype=np.int64)
        fxc = fx[o_c]
        fyc = fy[o_c]

        dm = rank < Kc
        dlid, dr = lid[dm], rank[dm]
        if REPL[l]:
            p = dr % 128
            fo = (dr // 128) * npc + dlid
        else:
            p = dlid % 128
            fo = (dlid // 128) * Kc + dr
        dpos = p * spp + fo
        frcd_a = np.zeros(128 * spp * 4, dtype=np.float16)
        b4 = dpos * 4
        frcd_a[b4] = fxc[dm]
        frcd_a[b4 + 1] = fxc[dm]
        frcd_a[b4 + 2] = fyc[dm]
        frcd_a[b4 + 3] = fyc[dm]

        # overflow
        ovm = ~dm
        olid = lid[ovm]
        orank = rank[ovm] - Kc
        is_g = (orank % Ko) == 0
        gidx = np.cumsum(is_g) - 1
        glid = olid[is_g]
        seg_of_g = (glid >> 15).astype(np.int64)
        gs = np.searchsorted(seg_of_g, np.arange(len(caps)))
        gs = np.append(gs, len(seg_of_g))
        if np.any(np.diff(gs) > np.asarray(caps)):
            raise RuntimeError(
                f"ovf overflow LOD{l} core{c}: {np.diff(gs)} caps {caps}")
        base = np.concatenate([[0], np.cumsum(caps)])[:-1]
        gpos = base[seg_of_g] + (np.arange(len(seg_of_g)) - gs[seg_of_g])
        idx_s = np.zeros(captot, dtype=np.int16)
        idx_s[gpos] = (glid & 32767).astype(np.int16)
        gp = gpos[gidx] if len(gidx) else np.zeros(0, dtype=np.int64)
        opos = (gp // 128) * Ko + orank % Ko + (gp % 128) * \
            (captot // 128 * Ko)
        osl = captot // 128 * Ko
        frco_a = np.zeros(128 * osl * 4, dtype=np.float16)
        b4 = opos * 4
        frco_a[b4] = fxc[ovm]
        frco_a[b4 + 1] = fxc[ovm]
        frco_a[b4 + 2] = fyc[ovm]
        frco_a[b4 + 3] = fyc[ovm]

        per_core.append({
            "frcd": frcd_a.reshape(128, spp * 4),
            "idxo": np.ascontiguousarray(
                np.tile(idx_s.reshape(-1, 16).T, (8, 1))),
            "frco": frco_a.reshape(128, osl * 4),
            "o_dense": o_c[dm], "pos_dense": dpos,
            "o_ovf": o_c[ovm], "pos_ovf": opos,
        })
    return per_core


def kernel(**inputs):
    x = np.asarray(inputs["x"], dtype=np.float32)
    assert x.shape == (N_POINTS, 2), x.shape

    qfs = [_quads(inputs[f"grid_{l}"], l) for l in range(NUM_LODS)]

    scale = 1.0
    for _attempt in range(4):
        cfg = _make_config(scale)
        key = tuple(cfg["captot"])
        if key not in _cache:
            _cache[key] = _build_program(cfg)
        nc, names = _cache[key]
        try:
            streams = [_streams(x, l, cfg) for l in range(NUM_LODS)]
            break
        except RuntimeError:
            scale *= 1.5
    else:
        raise RuntimeError("stream capacity overflow")

    in_maps = []
    for c in range(N_CORES):
        m = {}
        for l in range(NUM_LODS):
            m[names["tabd"][l]] = _dense_table(qfs[l], l, c)
            m[names["tabo"][l]] = _ovf_table(qfs[l], l, c)
            s = streams[l][c]
            m[names["frcd"][l]] = s["frcd"]
            m[names["idxo"][l]] = s["idxo"]
            m[names["frco"][l]] = s["frco"]
        in_maps.append(m)

    res = run_bass_kernel_spmd(nc, in_maps, core_ids=list(range(N_CORES)))

    out = np.empty((N_POINTS, NUM_LODS * FEAT), dtype=np.float32)
    for l in range(NUM_LODS):
        for c in range(N_CORES):
            s = streams[l][c]
            od = np.asarray(res.results[c][names["outd"][l]]).reshape(-1, 4)
            out[s["o_dense"], l * FEAT:(l + 1) * FEAT] = \
                od[s["pos_dense"]].astype(np.float32)
            if len(s["o_ovf"]):
                oo = np.asarray(
                    res.results[c][names["outo"][l]]).reshape(-1, 4)
                out[s["o_ovf"], l * FEAT:(l + 1) * FEAT] = \
                    oo[s["pos_ovf"]].astype(np.float32)
    return out


# revision 27
# speedup vs baseline: 1.0879x; 1.0720x over previous
"""DenseGrid multi-LOD bilinear embedding lookup on 8 Trainium2 NeuronCores.

Design: dense cell-order streaming + tiny gathered overflow.

Each LOD's quad table (per interior cell: [d0(4), e1(4), g00(4), e0(4)]
fp16 — value, x-difference, y-difference and cross term) is laid out
partition-interleaved on the host and STREAMED contiguously into SBUF — zero
gather descriptors. Every interior cell gets a fixed budget of Kc point
slots; the host sorts points by cell and scatters the first Kc points of
each cell into the dense slot stream (fx,fx,fy,fy fp16 per slot). Points
beyond Kc go to a small per-cell-group overflow pass that fetches quads via
SWDGE dma_gather exactly like the old design, but with ~6x fewer
descriptors (the old per-group gather of all points was Q7
descriptor-generation bound at ~4 ns/idx = 1.5 ms busy).

The bilinear is evaluated as six 4-element fp16 TENSOR_TENSOR ops
    ma = d0*fx; mb = e1*fx; ra = ma+g00; rb = mb+e0; out = ra + fy*rb
(quad re-encoded per cell as [d0, e1, g00, e0]) on DVE, with every
operand's innermost AP dim stride +-1 (fx/fy stored duplicated as pairs),
which engages the 2x TENSOR_TENSOR mode, and with every non-broadcast
operand a contiguous full tile (strided 4-of-8 reads measured ~1 el/cyc
vs ~2 contiguous).

Host does O(N) sort/index/fraction prep and O(table) re-layout only; all
per-point table-value movement happens on device.
"""
import math

import numpy as np
import concourse.bacc as bacc
import concourse.bass as bass
import concourse.mybir as mybir
import concourse.tile as tile
from concourse.bass_utils import run_bass_kernel_spmd

BASE_LOD = 4
NUM_LODS = 8
FEAT = 4
LODS = [2 ** L for L in range(BASE_LOD, BASE_LOD + NUM_LODS)]
N_POINTS = 2_000_000
N_CORES = 8
W = 32768                                # cells per overflow segment (int16)

IC = [(r - 1) * (r - 1) for r in LODS]   # interior cells per LOD
NPC = [-(-ic // N_CORES) for ic in IC]   # interior cells per core
KC = [8960, 2112, 512, 128, 36, 9, 4, 1]   # dense slots per cell
KO = [256, 256, 128, 64, 16, 8, 4, 2]       # overflow slots per group
REPL = [True, False, False, False, False, False, False, False]
NB = [NPC[l] if REPL[l] else -(-NPC[l] // 128) for l in range(NUM_LODS)]
# dense slots per partition
SPP = [NPC[l] * (KC[l] // 128) if REPL[l] else NB[l] * KC[l]
       for l in range(NUM_LODS)]
NSEG = [-(-NPC[l] // W) for l in range(NUM_LODS)]
OCOLS = [max(128, NSEG[l] * 16) for l in range(NUM_LODS)]  # ovf table cols
NI_MAX = 16384                           # idxs per gather instruction
CSMAX = 256                              # cell-rows per DVE sub-chunk
KRMAX = 256                              # k-range per DVE sub-chunk
TSLOT = 2048                             # slots/partition per DMA tile
SSLOT = 1024                             # slots/partition per DVE sub-chunk


def _ovf_moments(lam, kc, ko):
    """E and Var of ceil(max(n-kc,0)/ko) for n ~ Poisson(lam)."""
    kmax = int(lam + 10.0 * math.sqrt(lam) + 25 + kc)
    ks = np.arange(kmax + 1, dtype=np.float64)
    logfact = np.concatenate([[0.0], np.cumsum(np.log(ks[1:]))])
    pmf = np.exp(ks * math.log(lam) - lam - logfact)
    g = np.ceil(np.maximum(ks - kc, 0.0) / ko)
    e = float(np.sum(pmf * g))
    v = float(np.sum(pmf * g * g)) - e * e
    return e, max(v, 0.0)


def _make_config(scale=1.0):
    """Per-LOD per-segment overflow group capacities + gather instrs."""
    capg = []
    for l in range(NUM_LODS):
        lam = N_POINTS / IC[l]
        e1, v1 = _ovf_moments(lam, KC[l], KO[l])
        caps = []
        for s in range(NSEG[l]):
            ncell = min(W, NPC[l] - s * W)
            c_ = ncell * e1 + 6.0 * math.sqrt(max(ncell * v1, 1.0)) + 64
            caps.append(int(-(-(c_ * scale) // 128) * 128))
        capg.append(caps)
    instrs = []   # (lod, seg, group_offset, num_idxs)
    for l in range(NUM_LODS):
        off = 0
        for s in range(NSEG[l]):
            left = capg[l][s]
            while left:
                ni = min(NI_MAX, left)
                instrs.append((l, s, off, ni))
                off += ni
                left -= ni
    captot = [sum(caps) for caps in capg]
    return {"capg": capg, "captot": captot, "instrs": instrs}


def _raw_dma_gather(nc, out_ap, in_ap, idxs_ap, num_idxs, elem_size,
                    elem_step, queue_num):
    """dma_gather with elem_size_bytes below 256 (stride must be 256B mult)."""
    eng = nc.gpsimd
    stride_bytes = elem_step * mybir.dt.size(in_ap.dtype)
    assert stride_bytes % 256 == 0 and stride_bytes // 256 < 256
    assert in_ap.ap[0][0] == elem_step, in_ap.ap
    assert in_ap.ap[-1][1] == elem_size, in_ap.ap
    _in_ap = eng.lower_ap_dma(in_ap, for_custom_bir_dma=True)
    _idxs_ap = eng.lower_ap(idxs_ap)
    _out_ap = eng.lower_ap(out_ap)
    return eng.add_instruction(
        mybir.InstDMAGatherAnt(
            name=eng.bass.get_next_instruction_name(),
            ins=[*_in_ap, _idxs_ap,
                 eng.lower_val_access(eng.to_reg(num_idxs))],
            outs=[_out_ap],
            transpose=False, num_idxs=num_idxs, elem_size=elem_size,
            stride_bytes_256=stride_bytes // 256, gen_mode=0,
            single_packet=False, queue_num=queue_num,
            sbuf_tokens_per_rank=0, sbuf_free_dim_per_rank=0,
            sbuf_free_dim_pad_per_rank=0, sbuf_byte_offset=0,
        ))


def _emit_horner(nc, wk, qt, ft, ot, C, K, k_major, eng=None):
    """6-op fp16 bilinear applying per-cell quads to K packed slots.

    Quad layout per cell: [d0(4), e1(4), g00(4), e0(4)] with
    d0 = g01-g00, e0 = g10-g00, e1 = g11-g10-g01+g00:
        ma = d0*fx; mb = e1*fx; ra = ma+g00; rb = mb+e0
        out = ra + fy*rb
    Six 4-el ops instead of four 8/4-el ops: every non-broadcast operand
    is a full contiguous tile read/write (the 4-of-8 strided reads of the
    fused form measured ~1 elem/cycle vs ~2 for contiguous).
    qt: SBUF quad tile [128, C*16]; ft/ot: frc/out tiles, slot (c, k) at
    (c*K+k)*4 el, or (k*C+c)*4 when k_major (replicated-table LODs).
    frc per slot is (fx,fx,fy,fy) so every innermost AP dim is a stride-1
    pair, engaging the DVE 2x TENSOR_TENSOR mode; pre-merge 5D APs must
    stay mergeable to <= 3 free dims for the ISA.
    """
    eng = eng or nc.vector
    for k0 in range(0, K, KRMAX):
        kr = min(KRMAX, K - k0)
        n = C * kr
        tiles = [wk.tile([128, n * 4], mybir.dt.float16, tag=t, name=t)
                 for t in ("ma", "mb", "ra", "rb", "myt")]

        q3 = qt.rearrange("p (c e) -> p c e", e=16)
        if k_major:
            sl = slice(k0 * C * 4, (k0 + kr) * C * 4)
            f5 = ft[:, sl].rearrange("p (k c f b) -> p k c f b",
                                     c=C, f=2, b=2)
            o5 = ot[:, sl].rearrange("p (k c f b) -> p k c f b",
                                     c=C, f=2, b=2)
            def mk5(t):
                return t[:].rearrange("p (k c f b) -> p k c f b",
                                      c=C, f=2, b=2)
            def qb(lo):
                return q3[:, :, lo:lo + 4].rearrange(
                    "p c (f b) -> p c f b", b=2) \
                    .unsqueeze(1).broadcast_to([128, kr, C, 2, 2])
        else:
            if kr == K:
                f5 = ft.rearrange("p (c k f b) -> p c k f b",
                                  k=K, f=2, b=2)
                o5 = ot.rearrange("p (c k f b) -> p c k f b",
                                  k=K, f=2, b=2)
            else:
                # k-sliced views of a [c, K] layout only stay mergeable
                # to <= 3 free dims when there is a single cell row
                assert C == 1, (C, K)
                f5 = ft[:, k0 * 4:(k0 + kr) * 4].rearrange(
                    "p (c k f b) -> p c k f b", c=1, f=2, b=2)
                o5 = ot[:, k0 * 4:(k0 + kr) * 4].rearrange(
                    "p (c k f b) -> p c k f b", c=1, f=2, b=2)
            def mk5(t):
                return t[:].rearrange("p (c k f b) -> p c k f b",
                                      k=kr, f=2, b=2)
            def qb(lo):
                return q3[:, 0:C, lo:lo + 4].rearrange(
                    "p c (f b) -> p c f b", b=2) \
                    .unsqueeze(2).broadcast_to([128, C, kr, 2, 2])

        shp3 = [128, kr, C] if k_major else [128, C, kr]
        fx5 = f5[:, :, :, 0:1, :].broadcast_to(shp3 + [2, 2])
        fy5 = f5[:, :, :, 1:2, :].broadcast_to(shp3 + [2, 2])
        ma5, mb5, ra5, rb5, my5 = (mk5(t) for t in tiles)

        eng.tensor_mul(out=ma5, in0=qb(0), in1=fx5)
        eng.tensor_mul(out=mb5, in0=qb(4), in1=fx5)
        eng.tensor_add(out=ra5, in0=ma5, in1=qb(8))
        eng.tensor_add(out=rb5, in0=mb5, in1=qb(12))
        eng.tensor_mul(out=my5, in0=rb5, in1=fy5)
        eng.tensor_add(out=o5, in0=ra5, in1=my5)


def _build_program(cfg):
    captot = cfg["captot"]
    nc = bacc.Bacc(None, target_bir_lowering=False, num_swdge_queues=4)
    with tile.TileContext(nc) as tc:
        with tc.tile_pool(name="dram", bufs=1, space="DRAM") as dram, \
             tc.tile_pool(name="ov", bufs=1) as ov, \
             tc.tile_pool(name="qp", bufs=3) as qp, \
             tc.tile_pool(name="fp", bufs=2) as fp, \
             tc.tile_pool(name="op", bufs=2) as op, \
             tc.tile_pool(name="wk", bufs=1) as wk:
            tabd = [dram.tile([128, NB[l] * 16], mybir.dt.float16,
                              kind="ExternalInput", name=f"tabd_{l}")
                    for l in range(NUM_LODS)]
            frcd = [dram.tile([128, SPP[l] * 4], mybir.dt.float16,
                              kind="ExternalInput", name=f"frcd_{l}")
                    for l in range(NUM_LODS)]
            outd = [dram.tile([128, SPP[l] * 4], mybir.dt.float16,
                              kind="ExternalOutput", name=f"outd_{l}")
                    for l in range(NUM_LODS)]
            tabo = [dram.tile([min(W, NPC[l]), OCOLS[l]], mybir.dt.float16,
                              kind="ExternalInput", name=f"tabo_{l}")
                    for l in range(NUM_LODS)]
            idxo = [dram.tile([128, captot[l] // 16], mybir.dt.int16,
                              kind="ExternalInput", name=f"idxo_{l}")
                    for l in range(NUM_LODS)]
            frco = [dram.tile([128, captot[l] * KO[l] // 128 * 4],
                              mybir.dt.float16,
                              kind="ExternalInput", name=f"frco_{l}")
                    for l in range(NUM_LODS)]
            outo = [dram.tile([128, captot[l] * KO[l] // 128 * 4],
                              mybir.dt.float16,
                              kind="ExternalOutput", name=f"outo_{l}")
                    for l in range(NUM_LODS)]

            # ---- dense streamed chunk emitter ---------------------------
            def emit_dense(l):
                if REPL[l]:
                    # replicated quad table, stream order (k, c)
                    qt = qp.tile([128, NB[l] * 16], mybir.dt.float16,
                                 tag="qtr")
                    nc.sync.dma_start(out=qt[:], in_=tabd[l][:])
                    kcp = KC[l] // 128
                    C = NPC[l]
                    kchunk = max(1, SSLOT // C)
                    ktile = kchunk
                    for t0 in range(0, kcp, ktile):
                        tr = min(ktile, kcp - t0)
                        n = C * tr
                        ft = fp.tile([128, n * 4], mybir.dt.float16,
                                     tag="ft")
                        nc.scalar.dma_start(
                            out=ft[:],
                            in_=frcd[l][:, t0 * C * 4:(t0 + tr) * C * 4])
                        ot = op.tile([128, n * 4], mybir.dt.float16,
                                     tag="ot")
                        for k0 in range(0, tr, kchunk):
                            kr = min(kchunk, tr - k0)
                            _emit_horner(
                                nc, wk, qt[:],
                                ft[:, k0 * C * 4:(k0 + kr) * C * 4],
                                ot[:, k0 * C * 4:(k0 + kr) * C * 4],
                                C, kr, True)
                        nc.sync.dma_start(
                            out=outd[l][:, t0 * C * 4:(t0 + tr) * C * 4],
                            in_=ot[:])
                elif KC[l] > SSLOT:
                    # one cell row per partition, k-chunked (L1)
                    K = KC[l]
                    qt = qp.tile([128, 16], mybir.dt.float16, tag="qt1")
                    nc.sync.dma_start(out=qt[:], in_=tabd[l][:])
                    for k0 in range(0, K, TSLOT):
                        kb = min(TSLOT, K - k0)
                        ft = fp.tile([128, kb * 4], mybir.dt.float16,
                                     tag="ft")
                        nc.scalar.dma_start(
                            out=ft[:], in_=frcd[l][:, k0 * 4:(k0 + kb) * 4])
                        ot = op.tile([128, kb * 4], mybir.dt.float16,
                                     tag="ot")
                        _emit_horner(nc, wk, qt[:], ft[:], ot[:],
                                     1, kb, False)
                        nc.sync.dma_start(
                            out=outd[l][:, k0 * 4:(k0 + kb) * 4], in_=ot[:])
                else:
                    K = KC[l]
                    cchunk = 1 if K > KRMAX else \
                        max(1, min(CSMAX, SSLOT // K))
                    ctile = 2 * cchunk
                    for t0 in range(0, NB[l], ctile):
                        ts = min(ctile, NB[l] - t0)
                        qt = qp.tile([128, ts * 16], mybir.dt.float16,
                                     tag="qt")
                        nc.sync.dma_start(
                            out=qt[:],
                            in_=tabd[l][:, t0 * 16:(t0 + ts) * 16])
                        ft = fp.tile([128, ts * K * 4], mybir.dt.float16,
                                     tag="ft")
                        nc.scalar.dma_start(
                            out=ft[:],
                            in_=frcd[l][:, t0 * K * 4:(t0 + ts) * K * 4])
                        ot = op.tile([128, ts * K * 4], mybir.dt.float16,
                                     tag="ot")
                        for c0 in range(0, ts, cchunk):
                            cs = min(cchunk, ts - c0)
                            _emit_horner(
                                nc, wk,
                                qt[:, c0 * 16:(c0 + cs) * 16],
                                ft[:, c0 * K * 4:(c0 + cs) * K * 4],
                                ot[:, c0 * K * 4:(c0 + cs) * K * 4],
                                cs, K, False)
                        oeng = nc.scalar if (t0 // max(ctile, 1)) % 2 \
                            else nc.sync
                        oeng.dma_start(
                            out=outd[l][:, t0 * K * 4:(t0 + ts) * K * 4],
                            in_=ot[:])

            emit_dense(0)

            # ---- phase 1: overflow idx/frc loads + all gathers ----------
            it = []
            fot = []
            qot = []
            for l in range(NUM_LODS):
                t = ov.tile([128, captot[l] // 16], mybir.dt.int16,
                            tag=f"it{l}")
                nc.sync.dma_start(out=t[:], in_=idxo[l][:])
                it.append(t)
                t = ov.tile([128, captot[l] * KO[l] // 128 * 4],
                            mybir.dt.float16, tag=f"fo{l}")
                nc.scalar.dma_start(out=t[:], in_=frco[l][:])
                fot.append(t)
                qo_t = ov.tile([128, captot[l] // 128 * 16],
                               mybir.dt.float16, tag=f"qo{l}",
                               name=f"qo{l}")
                qot.append(qo_t)
            qn = 0
            for (l, s, off, ni) in cfg["instrs"]:
                _raw_dma_gather(
                    nc,
                    out_ap=qot[l][:, off // 128 * 16:(off + ni) // 128 * 16]
                    .rearrange("p (c e) -> p c e", e=16),
                    in_ap=tabo[l][:][:, 16 * s:16 * s + 16],
                    idxs_ap=it[l][:, off // 16:(off + ni) // 16],
                    num_idxs=ni, elem_size=16, elem_step=OCOLS[l],
                    queue_num=qn % 4)
                qn += 1

            for l in range(1, NUM_LODS):
                emit_dense(l)

            # ---- phase 3: overflow applies (gpsimd, own pools) ----------
            for l in range(NUM_LODS):
                rows = captot[l] // 128
                K = KO[l]
                cchunk = max(1, min(CSMAX, 1024 // K))
                for c0 in range(0, rows, cchunk):
                    cs = min(cchunk, rows - c0)
                    ot = op.tile([128, cs * K * 4], mybir.dt.float16,
                                 tag="ot")
                    _emit_horner(nc, wk,
                                 qot[l][:, c0 * 16:(c0 + cs) * 16],
                                 fot[l][:, c0 * K * 4:(c0 + cs) * K * 4],
                                 ot[:], cs, K, False)
                    nc.scalar.dma_start(
                        out=outo[l][:, c0 * K * 4:(c0 + cs) * K * 4],
                        in_=ot[:])
    nc.compile()
    names = {
        "tabd": [t.name for t in tabd], "frcd": [t.name for t in frcd],
        "outd": [t.name for t in outd], "tabo": [t.name for t in tabo],
        "idxo": [t.name for t in idxo], "frco": [t.name for t in frco],
        "outo": [t.name for t in outo],
    }
    return nc, names


_cache = {}


def _quads(g, l):
    """Interior-cell quad array [(res-1)^2, 16] fp16."""
    res = LODS[l]
    g3 = np.asarray(g, dtype=np.float32).reshape(res, res, FEAT)
    q = np.empty((res - 1, res - 1, 16), dtype=np.float16)
    d0 = g3[:-1, 1:] - g3[:-1, :-1]
    q[:, :, 0:4] = d0
    q[:, :, 4:8] = (g3[1:, 1:] - g3[1:, :-1]) - d0
    q[:, :, 8:12] = g3[:-1, :-1]
    q[:, :, 12:16] = g3[1:, :-1] - g3[:-1, :-1]
    return q.reshape(-1, 16)


def _dense_table(qf, l, c):
    """Per-core dense streamed table [128, NB*16]."""
    npc = NPC[l]
    sl = qf[c * npc:min((c + 1) * npc, IC[l])]
    if REPL[l]:
        flat = np.zeros(npc * 16, dtype=np.float16)
        flat[:sl.size] = sl.reshape(-1)
        return np.ascontiguousarray(
            np.broadcast_to(flat[None, :], (128, npc * 16)))
    nb = NB[l]
    arr = np.zeros((nb * 128, 16), dtype=np.float16)
    arr[:len(sl)] = sl
    return np.ascontiguousarray(
        arr.reshape(nb, 128, 16).transpose(1, 0, 2)).reshape(128, nb * 16)


def _ovf_table(qf, l, c):
    """Per-core overflow gather table [min(W,NPC), OCOLS]."""
    npc = NPC[l]
    sl = qf[c * npc:min((c + 1) * npc, IC[l])]
    rows = min(W, npc)
    out = np.zeros((rows, OCOLS[l]), dtype=np.float16)
    for s in range(NSEG[l]):
        seg = sl[s * W:(s + 1) * W]
        out[:len(seg), 16 * s:16 * s + 16] = seg
    return out


def _streams(x, l, cfg):
    """Host: per-core dense + overflow streams for LOD l."""
    res = LODS[l]
    R1 = res - 1
    Kc, Ko = KC[l], KO[l]
    npc = NPC[l]
    caps = cfg["capg"][l]
    captot = int(np.sum(caps))
    spp = SPP[l]
    xs = x[:, 0] * np.float32(R1)
    ys = x[:, 1] * np.float32(R1)
    hi = np.float32(R1 - 1e-05)
    x1 = np.floor(np.clip(xs, 0, hi)).astype(np.int32)
    y1 = np.floor(np.clip(ys, 0, hi)).astype(np.int32)
    fx = (xs - x1.astype(np.float32)).astype(np.float16)
    fy = (ys - y1.astype(np.float32)).astype(np.float16)
    ic = y1 * R1 + x1
    order = np.argsort(ic, kind="stable")
    sic = ic[order]
    cb = np.searchsorted(sic, np.arange(N_CORES + 1, dtype=np.int64) * npc)
    per_core = []
    for c in range(N_CORES):
        o_c = order[cb[c]:cb[c + 1]]
        lid = (sic[cb[c]:cb[c + 1]] - c * npc).astype(np.int64)
        n = len(lid)
        if n:
            newc = np.empty(n, dtype=bool)
            newc[0] = True
            newc[1:] = lid[1:] != lid[:-1]
            run_start = np.maximum.accumulate(
                np.where(newc, np.arange(n), 0))
            rank = np.arange(n) - run_start
        else:
            rank = np.zeros(0, dtype=np.int64)
        fxc = fx[o_c]
        fyc = fy[o_c]

        dm = rank < Kc
        dlid, dr = lid[dm], rank[dm]
        if REPL[l]:
            p = dr % 128
            fo = (dr // 128) * npc + dlid
        else:
            p = dlid % 128
            fo = (dlid // 128) * Kc + dr
        dpos = p * spp + fo
        frcd_a = np.zeros(128 * spp * 4, dtype=np.float16)
        b4 = dpos * 4
        frcd_a[b4] = fxc[dm]
        frcd_a[b4 + 1] = fxc[dm]
        frcd_a[b4 + 2] = fyc[dm]
        frcd_a[b4 + 3] = fyc[dm]

        # overflow
        ovm = ~dm
        olid = lid[ovm]
        orank = rank[ovm] - Kc
        is_g = (orank % Ko) == 0
        gidx = np.cumsum(is_g) - 1
        glid = olid[is_g]
        seg_of_g = (glid >> 15).astype(np.int64)
        gs = np.searchsorted(seg_of_g, np.arange(len(caps)))
        gs = np.append(gs, len(seg_of_g))
        if np.any(np.diff(gs) > np.asarray(caps)):
            raise RuntimeError(
                f"ovf overflow LOD{l} core{c}: {np.diff(gs)} caps {caps}")
        base = np.concatenate([[0], np.cumsum(caps)])[:-1]
        gpos = base[seg_of_g] + (np.arange(len(seg_of_g)) - gs[seg_of_g])
        idx_s = np.zeros(captot, dtype=np.int16)
        idx_s[gpos] = (glid & 32767).astype(np.int16)
        gp = gpos[gidx] if len(gidx) else np.zeros(0, dtype=np.int64)
        opos = (gp // 128) * Ko + orank % Ko + (gp % 128) * \
            (captot // 128 * Ko)
        osl = captot // 128 * Ko
        frco_a = np.zeros(128 * osl * 4, dtype=np.float16)
        b4 = opos * 4
        frco_a[b4] = fxc[ovm]
        frco_a[b4 + 1] = fxc[ovm]
        frco_a[b4 + 2] = fyc[ovm]
        frco_a[b4 + 3] = fyc[ovm]

        per_core.append({
            "frcd": frcd_a.reshape(128, spp * 4),
            "idxo": np.ascontiguousarray(
                np.tile(idx_s.reshape(-1, 16).T, (8, 1))),
            "frco": frco_a.reshape(128, osl * 4),
            "o_dense": o_c[dm], "pos_dense": dpos,
            "o_ovf": o_c[ovm], "pos_ovf": opos,
        })
    return per_core


def kernel(**inputs):
    x = np.asarray(inputs["x"], dtype=np.float32)
    assert x.shape == (N_POINTS, 2), x.shape

    qfs = [_quads(inputs[f"grid_{l}"], l) for l in range(NUM_LODS)]

    scale = 1.0
    for _attempt in range(4):
        cfg = _make_config(scale)
        key = tuple(cfg["captot"])
        if key not in _cache:
            _cache[key] = _build_program(cfg)
        nc, names = _cache[key]
        try:
            streams = [_streams(x, l, cfg) for l in range(NUM_LODS)]
            break
        except RuntimeError:
            scale *= 1.5
    else:
        raise RuntimeError("stream capacity overflow")

    in_maps = []
    for c in range(N_CORES):
        m = {}
        for l in range(NUM_LODS):
            m[names["tabd"][l]] = _dense_table(qfs[l], l, c)
            m[names["tabo"][l]] = _ovf_table(qfs[l], l, c)
            s = streams[l][c]
            m[names["frcd"][l]] = s["frcd"]
            m[names["idxo"][l]] = s["idxo"]
            m[names["frco"][l]] = s["frco"]
        in_maps.append(m)

    res = run_bass_kernel_spmd(nc, in_maps, core_ids=list(range(N_CORES)))

    out = np.empty((N_POINTS, NUM_LODS * FEAT), dtype=np.float32)
    for l in range(NUM_LODS):
        for c in range(N_CORES):
            s = streams[l][c]
            od = np.asarray(res.results[c][names["outd"][l]]).reshape(-1, 4)
            out[s["o_dense"], l * FEAT:(l + 1) * FEAT] = \
                od[s["pos_dense"]].astype(np.float32)
            if len(s["o_ovf"]):
                oo = np.asarray(
                    res.results[c][names["outo"][l]]).reshape(-1, 4)
                out[s["o_ovf"], l * FEAT:(l + 1) * FEAT] = \
                    oo[s["pos_ovf"]].astype(np.float32)
    return out
